# revision 1
# baseline (speedup 1.0000x reference)
"""Trainium2 Bass kernel for nn_ClusteringLayer (vq codebook assign + gather).

Math (per reference): for each token t, idx = argmin_k ||c_k||^2 - 2 x_t . c_k,
y_t = centers[idx]. Output = stack([x, y]).

Strategy: data-parallel over tokens across 8 NeuronCores (batch axis shard,
codebook replicated). On each core, scores s = (2x).c - ||c||^2 are computed
on the PE with an exact bf16 hi/lo 3-term expansion (xh.ch + xh.cl + xl.ch,
fp32 PSUM accumulation), which reproduces fp32 argmin decisions for this
distribution (verified: 0 argmin flips vs fp64 on the full input set, while
1-pass bf16 flips 135). Per 128-token tile: 8 PSUM banks of [128,512] scores,
DVE evacuates (psum - c2) to SBUF, a max/max_index pair finds the argmax
column, and an indirect DMA gathers centers rows into y.
"""

import numpy as np
import ml_dtypes

import concourse.bass as bass
import concourse.bacc as bacc
import concourse.mybir as mybir
import concourse.tile as tile
from concourse.bass_utils import run_bass_kernel_spmd

B, T, D, K = 8, 4096, 512, 4096
NCORES = 8
TOK = (B * T) // NCORES      # tokens per core
P = 128                      # partitions / tokens per tile
NBANK = K // 512             # psum banks per token tile (8)
DCH = D // P                 # contraction chunks (4)
NEG_INF = -3.0e38

_PROGRAM_CACHE = {}

# test.py introspection: holds the BassKernelResults of the last run
LAST_RUN = {}


def _build_program(ttiles):
    dt = mybir.dt
    nc = bacc.Bacc("TRN2", target_bir_lowering=False, debug=False,
                   num_devices=NCORES)
    ntok = ttiles * P
    xh_d = nc.dram_tensor("xh", [D, ntok], dt.bfloat16, kind="ExternalInput").ap()
    xl_d = nc.dram_tensor("xl", [D, ntok], dt.bfloat16, kind="ExternalInput").ap()
    ch_d = nc.dram_tensor("ch", [D, K], dt.bfloat16, kind="ExternalInput").ap()
    cl_d = nc.dram_tensor("cl", [D, K], dt.bfloat16, kind="ExternalInput").ap()
    c2_d = nc.dram_tensor("c2", [P, K], dt.float32, kind="ExternalInput").ap()
    cent_d = nc.dram_tensor("cent", [K, D], dt.float32, kind="ExternalInput").ap()
    y_d = nc.dram_tensor("y", [ntok, D], dt.float32, kind="ExternalOutput").ap()

    with tile.TileContext(nc) as tc:
        with tc.tile_pool(name="const", bufs=1) as cpool, \
             tc.tile_pool(name="work", bufs=2) as wpool, \
             tc.tile_pool(name="psum", bufs=1, space="PSUM") as ppool:
            def load_x_tile(t):
                xh_t = wpool.tile([P, DCH, P], dt.bfloat16, tag="xh",
                                  name=f"xh{t}", bufs=3)
                nc.sync.dma_start(
                    out=xh_t,
                    in_=xh_d[:, t * P:(t + 1) * P].rearrange(
                        "(c p) f -> p c f", p=P))
                xl_t = wpool.tile([P, DCH, P], dt.bfloat16, tag="xl",
                                  name=f"xl{t}", bufs=3)
                nc.sync.dma_start(
                    out=xl_t,
                    in_=xl_d[:, t * P:(t + 1) * P].rearrange(
                        "(c p) f -> p c f", p=P))
                return xh_t, xl_t

            # x tiles for the first two iterations load ahead of the bulky
            # codebook preload so bank-0 compute is not queued behind it
            x_pre = {t: load_x_tile(t) for t in range(min(2, ttiles))}

            # PE warmup: dense N=512 matmuls on the (tiny, early) t=0 x tile
            # keep the PE busy while the codebook streams in, so the HAM
            # clock-gate is released (2.4 GHz) before the real stream starts.
            # Results are garbage and never read; bank slot ps7 is needed
            # last by the real tile-0 work, so no WAR stall.
            ps_warm = ppool.tile([P, 512], dt.float32, tag="ps7",
                                 name="pswarm")
            warm_src = x_pre[0][0]
            for w in range(16):
                nc.tensor.matmul(ps_warm, lhsT=warm_src[:, 0, :],
                                 rhs=warm_src, start=True, stop=True)

            # Preload codebook tiles. Bank 0/1 are column-sliced so their
            # matmuls can start after ~1 MB; the rest loads coarsely. The
            # dma_start instructions alternate between the Sync and the
            # (otherwise idle) Scalar sequencer: descriptor generation costs
            # ~0.8us per instruction, which throttles the head if one
            # sequencer issues everything.
            ch_sb = []
            cl_sb = []
            for d in range(DCH):
                t_ch = cpool.tile([P, K], dt.bfloat16, tag=f"ch{d}", name=f"ch{d}")
                ch_sb.append(t_ch)
                t_cl = cpool.tile([P, K], dt.bfloat16, tag=f"cl{d}", name=f"cl{d}")
                cl_sb.append(t_cl)
            c2_sb = cpool.tile([P, K], dt.float32, tag="c2", name="c2sb")
            eng = [nc.sync, nc.scalar]
            ei = 0
            col_groups = [slice(0, 512), slice(512, 1024), slice(1024, K)]
            for cols in col_groups:
                for d in range(DCH):
                    eng[ei % 2].dma_start(out=ch_sb[d][:, cols],
                                          in_=ch_d[d * P:(d + 1) * P, cols])
                    ei += 1
                    eng[ei % 2].dma_start(out=cl_sb[d][:, cols],
                                          in_=cl_d[d * P:(d + 1) * P, cols])
                    ei += 1
                eng[ei % 2].dma_start(out=c2_sb[:, cols], in_=c2_d[:, cols])
                ei += 1

            for t in range(ttiles):
                if t in x_pre:
                    xh_t, xl_t = x_pre.pop(t)
                else:
                    xh_t, xl_t = load_x_tile(t)

                scores = wpool.tile([P, K], dt.float32, tag="scores",
                                    name=f"sc{t}", bufs=2)
                maxh1 = wpool.tile([P, 8], dt.float32, tag="maxh1",
                                   name=f"maxh1_{t}", bufs=2)
                maxh2 = wpool.tile([P, 8], dt.float32, tag="maxh2",
                                   name=f"maxh2_{t}", bufs=2)
                idxh1 = wpool.tile([P, 8], dt.uint32, tag="idxh1",
                                   name=f"idxh1_{t}", bufs=2)
                idxh2 = wpool.tile([P, 8], dt.uint32, tag="idxh2",
                                   name=f"idxh2_{t}", bufs=2)
                mask = wpool.tile([P, 1], dt.uint32, tag="mask",
                                  name=f"mask{t}", bufs=2)
                idxsel = wpool.tile([P, 1], dt.uint32, tag="idxsel",
                                    name=f"idxsel{t}", bufs=2)
                ytile = wpool.tile([P, D], dt.float32, tag="yt",
                                   name=f"yt{t}", bufs=3)
                half = NBANK // 2  # banks per argmax half

                for n in range(NBANK):
                    ps = ppool.tile([P, 512], dt.float32, tag=f"ps{n}",
                                    name=f"ps{t}_{n}")
                    first = True
                    for (xlo, clo) in ((0, 0), (0, 1), (1, 0)):
                        xt = xl_t if xlo else xh_t
                        csb = cl_sb if clo else ch_sb
                        for d in range(DCH):
                            nc.tensor.matmul(
                                ps,
                                lhsT=xt[:, d, :],
                                rhs=csb[d][:, n * 512:(n + 1) * 512],
                                start=first,
                                stop=(xlo == 1 and d == DCH - 1),
                            )
                            first = False
                    nc.vector.tensor_tensor(
                        out=scores[:, n * 512:(n + 1) * 512],
                        in0=ps,
                        in1=c2_sb[:, n * 512:(n + 1) * 512],
                        op=mybir.AluOpType.subtract,
                    )
                    if n == half - 1:
                        # first-half argmax overlaps banks 4-7 compute
                        nc.vector.max(out=maxh1, in_=scores[:, :half * 512])
                        nc.vector.max_index(out=idxh1, in_max=maxh1,
                                            in_values=scores[:, :half * 512])
                # second-half argmax + cross-half select
                nc.vector.max(out=maxh2, in_=scores[:, half * 512:])
                nc.vector.max_index(out=idxh2, in_max=maxh2,
                                    in_values=scores[:, half * 512:])
                nc.vector.tensor_scalar(
                    out=idxsel, in0=idxh2[:, 0:1], scalar1=half * 512,
                    scalar2=None, op0=mybir.AluOpType.add)
                nc.vector.tensor_tensor(
                    out=mask, in0=maxh1[:, 0:1], in1=maxh2[:, 0:1],
                    op=mybir.AluOpType.is_ge)
                nc.vector.copy_predicated(
                    out=idxsel, mask=mask, data=idxh1[:, 0:1])
                nc.gpsimd.indirect_dma_start(
                    out=ytile,
                    out_offset=None,
                    in_=cent_d,
                    in_offset=bass.IndirectOffsetOnAxis(ap=idxsel, axis=0),
                )
                nc.sync.dma_start(out=y_d[t * P:(t + 1) * P, :], in_=ytile)

    nc.compile()
    return nc


def _get_program(ttiles):
    if ttiles not in _PROGRAM_CACHE:
        _PROGRAM_CACHE[ttiles] = _build_program(ttiles)
    return _PROGRAM_CACHE[ttiles]


def _prep_inputs(x, centers, ntok_per_core, ncores):
    bf16 = ml_dtypes.bfloat16
    flat = np.ascontiguousarray(np.asarray(x, dtype=np.float32).reshape(-1, D))
    c = np.ascontiguousarray(np.asarray(centers, dtype=np.float32))

    ch = c.astype(bf16)
    cl = (c - ch.astype(np.float32)).astype(bf16)
    chT = np.ascontiguousarray(ch.T)
    clT = np.ascontiguousarray(cl.T)
    c2 = (c * c).sum(axis=-1, dtype=np.float32)
    c2b = np.ascontiguousarray(np.broadcast_to(c2[None, :], (P, K)))

    in_maps = []
    for i in range(ncores):
        xs = flat[i * ntok_per_core:(i + 1) * ntok_per_core]
        x2 = 2.0 * xs  # exact in fp32
        xh = x2.astype(bf16)
        xl = (x2 - xh.astype(np.float32)).astype(bf16)
        in_maps.append({
            "xh": np.ascontiguousarray(xh.T),
            "xl": np.ascontiguousarray(xl.T),
            "ch": chT,
            "cl": clT,
            "c2": c2b,
            "cent": c,
        })
    return in_maps


def kernel(x, centers):
    x = np.asarray(x, dtype=np.float32)
    nc = _get_program(TOK // P)
    in_maps = _prep_inputs(x, centers, TOK, NCORES)
    res = run_bass_kernel_spmd(nc, in_maps, core_ids=list(range(NCORES)))
    LAST_RUN["res"] = res
    y = np.concatenate([r["y"] for r in res.results], axis=0).reshape(x.shape)
    return np.stack([x, y], axis=0)



# revision 22
# speedup vs baseline: 1.2394x; 1.2394x over previous
"""Trainium2 Bass kernel for nn_ClusteringLayer (vq codebook assign + gather).

Math (per reference): for each token t, idx = argmin_k ||c_k||^2 - 2 x_t . c_k,
y_t = centers[idx]. Output = stack([x, y]).

Strategy: data-parallel over tokens across 8 NeuronCores. Single bf16 matmul
pass computes approximate scores s = 2x.c + (512 - ||c||^2) (bias folded into
the PE via a 2-row hi/lo bf16 matmul, exact to 2^-16). Activation engine
evacuates PSUM to fp16 selection scores; DVE MAX8/FIND_INDEX8 extract the
top-8 candidate columns per token; the top R are rescored EXACTLY in fp32
(fused tensor_tensor_reduce: sum(2x*c) - c2 with gathered fp32 centroid rows)
and the winner's row is indirect-gathered to the output. Empirically (host
study on the full input set) the true argmin is always within approx-rank 2
of the bf16 scores (margin to the 8th-best >= 3.2 vs per-column noise ~0.08),
so R=4 has large safety margin; exact-rescore noise ~8e-5 vs min true
top1-top2 gap 3.2e-4.
"""

import os
import numpy as np
import ml_dtypes

import concourse.bass as bass
import concourse.bacc as bacc
import concourse.mybir as mybir
import concourse.tile as tile
from concourse.bass_utils import run_bass_kernel_spmd

# feature flags. Defaults are the hardware-validated shipping config:
# - multi-row indirect gather returns wrong data on HW -> split gathers
# - InstTensorTensorReduce crashes the exec unit on HW -> STT+accum rescore
V_GATHER_SPLIT = os.environ.get("V_GATHER_SPLIT", "1") == "1"
V_BIAS8 = os.environ.get("V_BIAS8", "0") == "1"
V_FP32_SCORES = os.environ.get("V_FP32_SCORES", "0") == "1"
V_NO_ACT_ACCUM = os.environ.get("V_NO_ACT_ACCUM", "0") == "1"
V_NO_IOTA = os.environ.get("V_NO_IOTA", "0") == "1"
V_NO_WARM = os.environ.get("V_NO_WARM", "0") == "1"
V_NO_POOL_COMPUTE = os.environ.get("V_NO_POOL_COMPUTE", "0") == "1"
V_NO_ACT_COPY = os.environ.get("V_NO_ACT_COPY", "0") == "1"
V_NO_TTR = os.environ.get("V_NO_TTR", "0") == "1"
V_STT = os.environ.get("V_STT", "1") == "1"
V_ACT_DOT = os.environ.get("V_ACT_DOT", "0") == "1"

B, T, D, K = 8, 4096, 512, 4096
NCORES = 8
TOK = (B * T) // NCORES      # tokens per core
P = 128                      # partitions / tokens per tile
NBANK = K // 512             # psum banks per token tile (8)
DCH = D // P                 # contraction chunks (4)
R = 4                        # candidates rescored exactly per token
CAUG = 516                   # centers row + (-c2) + pad
NEG_INF = -3.0e38

_PROGRAM_CACHE = {}

# test.py introspection: holds the BassKernelResults of the last run
LAST_RUN = {}


def _build_program(ttiles):
    dt = mybir.dt
    fp16 = dt.float32 if V_FP32_SCORES else dt.float16
    nbias = 8 if V_BIAS8 else 2
    nc = bacc.Bacc("TRN2", target_bir_lowering=False, debug=False,
                   num_devices=NCORES)
    ntok = ttiles * P
    xh_d = nc.dram_tensor("xh", [D, ntok], dt.bfloat16, kind="ExternalInput").ap()
    xf_d = nc.dram_tensor("xf", [ntok, D], dt.float32, kind="ExternalInput").ap()
    ch_d = nc.dram_tensor("ch", [D, K], dt.bfloat16, kind="ExternalInput").ap()
    c2b_d = nc.dram_tensor("c2b", [8, K], dt.bfloat16, kind="ExternalInput").ap()
    one2_d = nc.dram_tensor("one2", [8, P], dt.bfloat16, kind="ExternalInput").ap()
    io8_d = nc.dram_tensor("io8", [P, 8], dt.float32, kind="ExternalInput").ap()
    caug_d = nc.dram_tensor("caug", [K, CAUG], dt.float32, kind="ExternalInput").ap()
    y_d = nc.dram_tensor("y", [ntok, D], dt.float32, kind="ExternalOutput").ap()

    _pool_alu = nc.vector if V_NO_POOL_COMPUTE else nc.gpsimd

    with tile.TileContext(nc) as tc:
        with tc.tile_pool(name="const", bufs=1) as cpool, \
             tc.tile_pool(name="work", bufs=2) as wpool, \
             tc.tile_pool(name="psum", bufs=1, space="PSUM") as ppool:

            def load_x_tile(t):
                xh_t = wpool.tile([P, DCH, P], dt.bfloat16, tag="xh",
                                  name=f"xh{t}", bufs=3)
                nc.sync.dma_start(
                    out=xh_t,
                    in_=xh_d[:, t * P:(t + 1) * P].rearrange(
                        "(c p) f -> p c f", p=P))
                xf_t = wpool.tile([P, D], dt.float32, tag="xf",
                                  name=f"xf{t}", bufs=4)
                nc.scalar.dma_start(out=xf_t, in_=xf_d[t * P:(t + 1) * P, :])
                return xh_t, xf_t

            # x tiles for the first two iterations load ahead of the bulky
            # codebook preload so bank-0 compute is not queued behind it
            x_pre = {t: load_x_tile(t) for t in range(min(2, ttiles))}

            # PE warmup: dense matmuls on the (tiny, early) t=0 x tile keep
            # the PE busy while the codebook streams in, releasing the HAM
            # clock-gate (2.4 GHz) before the real stream starts. Bank slot
            # ps7 is needed last by the real tile-0 work, so no WAR stall.
            if not V_NO_WARM:
                ps_warm = ppool.tile([P, 512], dt.float32, tag="ps7",
                                     name="pswarm")
                warm_src = x_pre[0][0]
                warm_rhs = warm_src.rearrange("p c f -> p (c f)")
                for w in range(16):
                    nc.tensor.matmul(ps_warm, lhsT=warm_src[:, 0, :],
                                     rhs=warm_rhs, start=True, stop=True)

            # constants
            one2 = cpool.tile([nbias, P], dt.bfloat16, tag="one2", name="one2")
            nc.sync.dma_start(out=one2, in_=one2_d[0:nbias, :])
            c2b = cpool.tile([nbias, K], dt.bfloat16, tag="c2b", name="c2b")
            nc.sync.dma_start(out=c2b, in_=c2b_d[0:nbias, :])
            iota8f = cpool.tile([P, 8], dt.float32, tag="iota8f", name="iota8f")
            if V_NO_IOTA:
                nc.sync.dma_start(out=iota8f, in_=io8_d)
            else:
                iota8 = cpool.tile([P, 8], dt.int32, tag="iota8i", name="iota8i")
                nc.gpsimd.iota(iota8, pattern=[[1, 8]], base=0,
                               channel_multiplier=0)
                nc.gpsimd.tensor_copy(out=iota8f, in_=iota8)

            # codebook preload: column-sliced so bank-0 matmuls can start
            # after ~1 MB; dma_start instructions alternate between the Sync
            # and Scalar sequencers (descriptor generation ~0.8us each).
            ch_sb = [cpool.tile([P, K], dt.bfloat16, tag=f"ch{d}", name=f"ch{d}")
                     for d in range(DCH)]
            eng = [nc.sync, nc.scalar]
            ei = 0
            col_groups = [slice(0, 512), slice(512, 1024), slice(1024, 2048),
                          slice(2048, K)]
            for cols in col_groups:
                for d in range(DCH):
                    eng[ei % 2].dma_start(out=ch_sb[d][:, cols],
                                          in_=ch_d[d * P:(d + 1) * P, cols])
                    ei += 1

            # ---------------- pipeline stages ----------------
            state = {}   # t -> per-tile tiles

            def stage_compute(t):
                """PE matmuls (bias + 1-pass bf16) + Act evac to fp16."""
                if t in x_pre:
                    xh_t, xf_t = x_pre.pop(t)
                else:
                    xh_t, xf_t = load_x_tile(t)
                s16 = wpool.tile([P, K], fp16, tag="s16", name=f"s16_{t}",
                                 bufs=3)
                for n in range(NBANK):
                    ps = ppool.tile([P, 512], dt.float32, tag=f"ps{n}",
                                    name=f"ps{t}_{n}")
                    cols = slice(n * 512, (n + 1) * 512)
                    nc.tensor.matmul(ps, lhsT=one2, rhs=c2b[:, cols],
                                     start=True, stop=False)
                    for d in range(DCH):
                        nc.tensor.matmul(ps, lhsT=xh_t[:, d, :],
                                         rhs=ch_sb[d][:, cols],
                                         start=False, stop=(d == DCH - 1))
                    if V_NO_ACT_COPY:
                        nc.vector.tensor_copy(out=s16[:, cols], in_=ps)
                    else:
                        nc.scalar.copy(out=s16[:, cols], in_=ps)
                state[t] = {"s16": s16, "xf": xf_t}

            def stage_scan(t):
                """DVE top-8 scan + Pool candidate gather."""
                st = state[t]
                s16 = st["s16"]
                m8 = wpool.tile([P, 8], fp16, tag="m8", name=f"m8_{t}", bufs=2)
                i8 = wpool.tile([P, 8], dt.uint32, tag="i8", name=f"i8_{t}",
                                bufs=3)
                nc.vector.max(out=m8, in_=s16)
                nc.vector.max_index(out=i8, in_max=m8, in_values=s16)
                v8 = wpool.tile([P, 8], dt.float32, tag="v8", name=f"v8_{t}",
                                bufs=3)
                _pool_alu.memset(v8, NEG_INF)
                g = wpool.tile([P, R, CAUG], dt.float32, tag="g",
                               name=f"g{t}", bufs=3)
                if V_GATHER_SPLIT:
                    for r in range(R):
                        nc.gpsimd.indirect_dma_start(
                            out=g[:, r, :], out_offset=None, in_=caug_d,
                            in_offset=bass.IndirectOffsetOnAxis(
                                ap=i8[:, r:r + 1], axis=0))
                else:
                    nc.gpsimd.indirect_dma_start(
                        out=g, out_offset=None, in_=caug_d,
                        in_offset=bass.IndirectOffsetOnAxis(ap=i8[:, 0:R],
                                                            axis=0))
                st["i8"] = i8
                st["v8"] = v8
                st["g"] = g

            def stage_rescore(t):
                """DVE exact fp32 rescore of R candidates, winner select,
                Pool y-gather, DMA out."""
                st = state.pop(t)
                g, v8, i8, xf_t = st["g"], st["v8"], st["i8"], st["xf"]
                for r in range(R):
                    prod = wpool.tile([P, D], dt.float32, tag="prod",
                                      name=f"prod{t}_{r}", bufs=2)
                    if V_STT:
                        # v8[r] = sum(g_r * 2x); -c2 added afterwards
                        nc.vector.scalar_tensor_tensor(
                            out=prod, in0=g[:, r, 0:D], scalar=1.0,
                            in1=xf_t, op0=mybir.AluOpType.mult,
                            op1=mybir.AluOpType.mult,
                            accum_out=v8[:, r:r + 1])
                    elif V_ACT_DOT:
                        nc.vector.tensor_tensor(
                            out=prod, in0=g[:, r, 0:D], in1=xf_t,
                            op=mybir.AluOpType.mult)
                        junk2 = wpool.tile([P, D], dt.float32, tag="junk2",
                                           name=f"junk2_{t}_{r}", bufs=2)
                        nc.scalar.activation(
                            out=junk2, in_=prod,
                            func=mybir.ActivationFunctionType.Copy,
                            accum_out=v8[:, r:r + 1])
                    elif V_NO_TTR:
                        nc.vector.tensor_tensor(
                            out=prod, in0=g[:, r, 0:D], in1=xf_t,
                            op=mybir.AluOpType.mult)
                        dots = wpool.tile([P, 1], dt.float32, tag="dots",
                                          name=f"dots{t}_{r}", bufs=2)
                        nc.vector.tensor_reduce(
                            out=dots, in_=prod, axis=mybir.AxisListType.X,
                            op=mybir.AluOpType.add)
                        nc.vector.tensor_tensor(
                            out=v8[:, r:r + 1], in0=dots,
                            in1=g[:, r, D:D + 1], op=mybir.AluOpType.add)
                    else:
                        nc.vector.tensor_tensor_reduce(
                            out=prod, in0=g[:, r, 0:D], in1=xf_t, scale=1.0,
                            scalar=g[:, r, D:D + 1],
                            op0=mybir.AluOpType.mult, op1=mybir.AluOpType.add,
                            accum_out=v8[:, r:r + 1])
                if V_STT or V_ACT_DOT:
                    # v8[:, 0:R] += (-c2) of each candidate, one strided add
                    nc.vector.tensor_tensor(
                        out=v8[:, 0:R], in0=v8[:, 0:R], in1=g[:, 0:R, D],
                        op=mybir.AluOpType.add)
                vm8 = wpool.tile([P, 8], dt.float32, tag="vm8",
                                 name=f"vm8_{t}", bufs=2)
                pos8 = wpool.tile([P, 8], dt.uint32, tag="pos8",
                                  name=f"pos8_{t}", bufs=2)
                nc.vector.max(out=vm8, in_=v8)
                nc.vector.max_index(out=pos8, in_max=vm8, in_values=v8)
                # winner centroid index = sum_r i8[r] * (iota8 == pos)
                posf = wpool.tile([P, 1], dt.float32, tag="posf",
                                  name=f"posf{t}", bufs=2)
                _pool_alu.tensor_copy(out=posf, in_=pos8[:, 0:1])
                mask8 = wpool.tile([P, 8], dt.float32, tag="mask8",
                                   name=f"mask8_{t}", bufs=2)
                _pool_alu.tensor_scalar(out=mask8, in0=iota8f, scalar1=posf,
                                        scalar2=None,
                                        op0=mybir.AluOpType.is_equal)
                i8f = wpool.tile([P, 8], dt.float32, tag="i8f",
                                 name=f"i8f{t}", bufs=2)
                _pool_alu.tensor_copy(out=i8f, in_=i8)
                wi8 = wpool.tile([P, 8], dt.float32, tag="wi8",
                                 name=f"wi8_{t}", bufs=2)
                _pool_alu.tensor_tensor(out=wi8, in0=i8f, in1=mask8,
                                        op=mybir.AluOpType.mult)
                wif = wpool.tile([P, 1], dt.float32, tag="wif",
                                 name=f"wif{t}", bufs=2)
                if V_NO_ACT_ACCUM:
                    nc.vector.tensor_reduce(out=wif, in_=wi8,
                                            axis=mybir.AxisListType.X,
                                            op=mybir.AluOpType.add)
                else:
                    junk = wpool.tile([P, 8], dt.float32, tag="junk",
                                      name=f"junk{t}", bufs=2)
                    nc.scalar.activation(
                        out=junk, in_=wi8,
                        func=mybir.ActivationFunctionType.Copy,
                        accum_out=wif)
                wi = wpool.tile([P, 1], dt.uint32, tag="wi", name=f"wi{t}",
                                bufs=2)
                _pool_alu.tensor_copy(out=wi, in_=wif)
                yt = wpool.tile([P, CAUG], dt.float32, tag="yt",
                                name=f"yt{t}", bufs=2)
                nc.gpsimd.indirect_dma_start(
                    out=yt, out_offset=None, in_=caug_d,
                    in_offset=bass.IndirectOffsetOnAxis(ap=wi, axis=0))
                nc.sync.dma_start(out=y_d[t * P:(t + 1) * P, :],
                                  in_=yt[:, 0:D])

            for it in range(ttiles + 2):
                if it >= 2:
                    stage_rescore(it - 2)
                if 1 <= it <= ttiles:
                    stage_scan(it - 1)
                if it < ttiles:
                    stage_compute(it)

    nc.compile()
    return nc


def _get_program(ttiles):
    if ttiles not in _PROGRAM_CACHE:
        _PROGRAM_CACHE[ttiles] = _build_program(ttiles)
    return _PROGRAM_CACHE[ttiles]


def _prep_inputs(x, centers, ntok_per_core, ncores):
    bf16 = ml_dtypes.bfloat16
    flat = np.ascontiguousarray(np.asarray(x, dtype=np.float32).reshape(-1, D))
    c = np.ascontiguousarray(np.asarray(centers, dtype=np.float32))

    chT = np.ascontiguousarray(c.T.astype(bf16))
    c2 = (c.astype(np.float64) ** 2).sum(axis=-1)
    bias = (512.0 - c2).astype(np.float32)          # selection bias (shifted)
    bh = bias.astype(bf16)
    bl = (bias - bh.astype(np.float32)).astype(bf16)
    c2b = np.zeros((8, K), dtype=bf16)
    c2b[0] = bh
    c2b[1] = bl
    one2 = np.zeros((8, P), dtype=bf16)
    one2[0:2] = 1.0
    io8 = np.broadcast_to(np.arange(8, dtype=np.float32)[None, :],
                          (P, 8)).copy()
    caug = np.zeros((K, CAUG), dtype=np.float32)
    caug[:, :D] = c
    caug[:, D] = (-c2).astype(np.float32)           # exact-rescore bias

    in_maps = []
    for i in range(ncores):
        xs = flat[i * ntok_per_core:(i + 1) * ntok_per_core]
        x2 = 2.0 * xs  # exact in fp32
        in_maps.append({
            "xh": np.ascontiguousarray(x2.astype(bf16).T),
            "xf": np.ascontiguousarray(x2),
            "ch": chT,
            "c2b": c2b,
            "one2": one2,
            "io8": io8,
            "caug": caug,
        })
    return in_maps


def kernel(x, centers):
    x = np.asarray(x, dtype=np.float32)
    nc = _get_program(TOK // P)
    in_maps = _prep_inputs(x, centers, TOK, NCORES)
    res = run_bass_kernel_spmd(nc, in_maps, core_ids=list(range(NCORES)))
    LAST_RUN["res"] = res
    y = np.concatenate([r["y"] for r in res.results], axis=0).reshape(x.shape)
    return np.stack([x, y], axis=0)
